# revision 46
# baseline (speedup 1.0000x reference)
"""Trainium2 Bass kernel for the dual-branch cross-attention module.

Computation (see the module's reference):
    q1,k1,v1 = split(x @ w_qkv1); q2,k2,v2 = split(y @ w_qkv2)   (B,H,L,D)
    a1 = softmax(1 - q1 k2^T / sqrt(D));  xo = a1 @ v1
    a2 = softmax(1 - q2 k1^T / sqrt(D));  yo = a2 @ v2
    out = (xo @ w_p1 + b_p1, yo @ w_p2 + b_p2)

Sharding: batch*heads across 8 cores. Core c handles batch b=c//2 and the
8-head slice h0=(c%2)*8. Each core computes its full LxL attention and a
partial output projection over its 512 channels; the host sums the two
partials per batch and adds the bias (softmax(1-z) == softmax(-z), so the
constant shift is dropped).

Self-contained: shapes/sharding hardcoded; imports only the system bass stack.
"""

import os
import sys
from contextlib import ExitStack

import numpy as np

for _p in ("/opt/trn_rl_repo", os.path.expanduser("~/.axon_site/_ro/trn_rl_repo")):
    if os.path.isdir(_p) and _p not in sys.path:
        sys.path.insert(0, _p)

import concourse.tile as tile
from concourse import bacc, mybir
from concourse.bass_utils import run_bass_kernel_spmd
from concourse.masks import make_identity

F32 = mybir.dt.float32
BF16 = mybir.dt.bfloat16
EXP = mybir.ActivationFunctionType.Exp

L = 1024          # sequence length
DIM = 1024        # model dim
D = 64            # head dim
SCALE = D ** -0.5
PROJ = 256        # projection out dim
NCORES = 8
PAIRS = 4         # head pairs per core (8 heads / 2)
KC = 8            # contraction chunks of 128 over DIM
MC = 8            # key-position chunks of 128 over L
LWIN = 512        # query-window (matmul free dim)
NLW = L // LWIN

W_NAMES = ("wq1", "wk1", "wv1", "wq2", "wk2", "wv2")  # x-side first (load order)


def _build_body(nc, tc, ins, outs, ctx):
    stage = ctx.enter_context(tc.tile_pool(name="stage", bufs=3))
    big = ctx.enter_context(tc.tile_pool(name="big", bufs=1))
    qkp = ctx.enter_context(tc.tile_pool(name="qkp", bufs=3))
    vpool = ctx.enter_context(tc.tile_pool(name="vp", bufs=1))
    ep = ctx.enter_context(tc.tile_pool(name="ep", bufs=5))
    onp = ctx.enter_context(tc.tile_pool(name="onp", bufs=1))
    smp = ctx.enter_context(tc.tile_pool(name="smp", bufs=3))
    outp = ctx.enter_context(tc.tile_pool(name="outp", bufs=3))
    mm_ps = ctx.enter_context(tc.tile_pool(name="mm_ps", bufs=2, space="PSUM"))
    st_ps = ctx.enter_context(tc.tile_pool(name="st_ps", bufs=2, space="PSUM"))
    pv_ps = ctx.enter_context(tc.tile_pool(name="pv_ps", bufs=2, space="PSUM"))

    # ---- load x/y, transpose on PE (fp32), cast to bf16 on the psum copy ----
    # xT[p, c, l] = x[l, c*128 + p]
    ident = big.tile([128, 128], BF16, tag="ident")
    make_identity(nc, ident)
    xT = big.tile([128, KC, L], BF16, tag="xT")
    yT = big.tile([128, KC, L], BF16, tag="yT")
    w_bf = {}

    def emit_w_load(nm):
        wt = big.tile([128, KC, 512], BF16, tag=nm)
        w_r = ins[nm].rearrange("(c p) n -> p c n", p=128)
        for c in range(KC):
            wst = stage.tile([128, 512], F32, tag="wst")
            nc.sync.dma_start(out=wst, in_=w_r[:, c, :])
            nc.vector.tensor_copy(out=wt[:, c, :], in_=wst)
        w_bf[nm] = wt

    for src_name, dstT in (("xb", xT), ("yb", yT)):
        src_r = ins[src_name].rearrange("(i p) d -> p i d", p=128)
        for i in range(L // 128):
            xst = stage.tile([128, DIM], F32, tag="xst")
            nc.sync.dma_start(out=xst, in_=src_r[:, i, :])
            xbf = stage.tile([128, DIM], BF16, tag="xbf")
            nc.scalar.copy(out=xbf, in_=xst)
            for j in range(KC):
                if j % 2 == 0:
                    tp = mm_ps.tile([128, 512], BF16, tag="mm")
                else:
                    tp = st_ps.tile([128, 1024], BF16, tag="st")
                nc.tensor.transpose(tp[:, 0:128], xbf[:, j * 128:(j + 1) * 128],
                                    ident)
                nc.vector.tensor_copy(out=dstT[:, j, i * 128:(i + 1) * 128],
                                      in_=tp[:, 0:128])
        if src_name == "xb":
            for nm in ("wq1", "wk1", "wv1"):
                emit_w_load(nm)

    # ---- remaining weights ----
    for nm in ("wq2", "wk2", "wv2"):
        emit_w_load(nm)
    wp_bf = {}
    for nm in ("wp1", "wp2"):
        wt = big.tile([128, PAIRS, PROJ], BF16, tag=nm)
        w_r = ins[nm].rearrange("(c p) n -> p c n", p=128)
        for c in range(PAIRS):
            wst = stage.tile([128, 512], F32, tag="wst")
            nc.sync.dma_start(out=wst[:, 0:PROJ], in_=w_r[:, c, :])
            nc.vector.tensor_copy(out=wt[:, c, :], in_=wst[:, 0:PROJ])
        wp_bf[nm] = wt

    onorm = {}  # (pair, branch) -> [128, L] bf16, rows 0:64 head A, 64:128 head B
    vaug = {}   # (pair, branch) -> [128, MC, 130] bf16

    def emit_qk(p):
        cols = slice(p * 128, (p + 1) * 128)
        qk = {}
        for nm, src in (("q1", xT), ("k1", xT), ("q2", yT), ("k2", yT)):
            dstT = qkp.tile([128, L], BF16, tag=nm)
            wt = w_bf["w" + nm]
            for lw in range(NLW):
                mm = mm_ps.tile([128, 512], F32, tag="mm")
                for c in range(KC):
                    nc.tensor.matmul(
                        mm, wt[:, c, cols], src[:, c, lw * LWIN:(lw + 1) * LWIN],
                        start=(c == 0), stop=(c == KC - 1),
                    )
                nc.scalar.copy(out=dstT[:, lw * LWIN:(lw + 1) * LWIN], in_=mm)
            qk[nm] = dstT
        return qk

    def emit_v():
        # v for all pairs: natural layout + ones columns (N=512 matmuls)
        for br, (nm, src) in enumerate((("wv1", xT), ("wv2", yT))):
            wt = w_bf[nm]
            for p in range(PAIRS):
                va = vpool.tile([128, MC, 130], BF16, tag=f"va_{p}_{br}")
                nc.vector.memset(va[:, :, 64:65], 1.0)
                nc.vector.memset(va[:, :, 129:130], 1.0)
                vaug[(p, br)] = va
            for lt in range(MC):
                mm = mm_ps.tile([128, 512], F32, tag="mm")
                for c in range(KC):
                    nc.tensor.matmul(
                        mm, src[:, c, lt * 128:(lt + 1) * 128], wt[:, c, :],
                        start=(c == 0), stop=(c == KC - 1),
                    )
                for p in range(PAIRS):
                    va = vaug[(p, br)]
                    nc.vector.tensor_copy(out=va[:, lt, 0:64],
                                          in_=mm[:, p * 128:p * 128 + 64])
                    nc.vector.tensor_copy(out=va[:, lt, 65:129],
                                          in_=mm[:, p * 128 + 64:(p + 1) * 128])

    def emit_proj(br):
        # partial output projection over this core's 512 channels
        wp_nm, out_nm = (("wp1", "p1"), ("wp2", "p2"))[br]
        wt = wp_bf[wp_nm]
        out_r = outs[out_nm].rearrange("(i p) n -> p i n", p=128)
        for lt in range(L // 128):
            tsl = slice(lt * 128, (lt + 1) * 128)
            mm = mm_ps.tile([128, 512], F32, tag="mm")
            for pp in range(PAIRS):
                nc.tensor.matmul(mm[:, 0:PROJ], onorm[(pp, br)][:, tsl],
                                 wt[:, pp, :], start=(pp == 0), stop=(pp == PAIRS - 1))
            ob = outp.tile([128, PROJ], F32, tag="ob")
            nc.vector.tensor_copy(out=ob, in_=mm[:, 0:PROJ])
            nc.sync.dma_start(out=out_r[:, lt, :], in_=ob)

    for p in range(PAIRS):
        qk = emit_qk(p)
        if p == 0:
            emit_v()

        # ---- attention, both branches ----
        for br in range(2):
            qT = qk["q1"] if br == 0 else qk["q2"]
            kT = qk["k2"] if br == 0 else qk["k1"]
            va = vaug[(p, br)]
            on = onp.tile([128, L], BF16, tag=f"on_{p}_{br}")
            onorm[(p, br)] = on
            for lw in range(NLW):
                lsl = slice(lw * LWIN, (lw + 1) * LWIN)
                pvA = pv_ps.tile([65, 512], F32, tag="pv")
                pvB = pv_ps.tile([65, 512], F32, tag="pv")
                for mc in range(MC):
                    msl = slice(mc * 128, (mc + 1) * 128)
                    st = st_ps.tile([128, 1024], F32, tag="st")
                    # S^T[m, l] for both heads (K=64, row-packed 0-63 / 64-127)
                    nc.tensor.matmul(st[:, 0:512], kT[0:64, msl], qT[0:64, lsl],
                                     start=True, stop=True)
                    nc.tensor.matmul(st[:, 512:1024], kT[64:128, msl],
                                     qT[64:128, lsl], start=True, stop=True)
                    # E = exp(-S/sqrt(D)); softmax(1-z) == softmax(-z)
                    e_t = ep.tile([128, 1024], BF16, tag="E")
                    nc.scalar.activation(out=e_t, in_=st, func=EXP, scale=-SCALE)
                    # O^T accum + row-sums via the ones column
                    nc.tensor.matmul(pvA, va[:, mc, 0:65], e_t[:, 0:512],
                                     start=(mc == 0), stop=(mc == MC - 1))
                    nc.tensor.matmul(pvB, va[:, mc, 65:130], e_t[:, 512:1024],
                                     start=(mc == 0), stop=(mc == MC - 1))
                # normalize: O^T * (1/rowsum) broadcast along partitions
                for head, pv in ((0, pvA), (1, pvB)):
                    ssum = smp.tile([1, 512], F32, tag="ssum")
                    nc.vector.tensor_copy(out=ssum, in_=pv[64:65, :])
                    pvo = smp.tile([64, 512], F32, tag="pvo")
                    nc.vector.tensor_copy(out=pvo, in_=pv[0:64, :])
                    sb = smp.tile([64, 512], F32, tag="sb")
                    nc.gpsimd.partition_broadcast(sb, ssum)
                    rb = smp.tile([64, 512], F32, tag="rb")
                    nc.vector.reciprocal_approx_fast(out=rb, in_=sb)
                    nc.vector.tensor_mul(
                        out=on[head * 64:(head + 1) * 64, lsl],
                        in0=pvo, in1=rb,
                    )



    # ---- output projection (partial over this core's 512 channels) ----
    for br in range(2):
        emit_proj(br)

def build():
    nc = bacc.Bacc("TRN2", target_bir_lowering=False, debug=False,
                   num_devices=NCORES)
    ins = {}
    for nm in ("xb", "yb"):
        ins[nm] = nc.dram_tensor(nm, [L, DIM], F32, kind="ExternalInput").ap()
    for nm in W_NAMES:
        ins[nm] = nc.dram_tensor(nm, [DIM, 512], F32, kind="ExternalInput").ap()
    for nm in ("wp1", "wp2"):
        ins[nm] = nc.dram_tensor(nm, [512, PROJ], F32, kind="ExternalInput").ap()
    outs = {}
    for nm in ("p1", "p2"):
        outs[nm] = nc.dram_tensor(nm, [L, PROJ], F32, kind="ExternalOutput").ap()
    with tile.TileContext(nc) as tc:
        with ExitStack() as ctx:
            _build_body(nc, tc, ins, outs, ctx)
    nc.compile()
    return nc


_NC_CACHE = None


def _get_nc():
    global _NC_CACHE
    if _NC_CACHE is None:
        _NC_CACHE = build()
    return _NC_CACHE


def make_in_maps(x, y, w_qkv1, w_qkv2, w_p1, w_p2):
    """Shard the full inputs: core c -> batch c//2, head-slice (c%2)*8."""
    in_maps = []
    for c in range(NCORES):
        b, half = divmod(c, 2)
        c0 = half * 512  # channel offset of this core's 8 heads
        m = {
            "xb": np.ascontiguousarray(x[b]),
            "yb": np.ascontiguousarray(y[b]),
            "wp1": np.ascontiguousarray(w_p1[c0:c0 + 512, :]),
            "wp2": np.ascontiguousarray(w_p2[c0:c0 + 512, :]),
        }
        for wsrc, names in ((w_qkv1, ("wq1", "wk1", "wv1")),
                            (w_qkv2, ("wq2", "wk2", "wv2"))):
            for j, nm in enumerate(names):
                base = j * DIM + c0
                m[nm] = np.ascontiguousarray(wsrc[:, base:base + 512])
        in_maps.append(m)
    return in_maps


def run_cores(in_maps, trace=False, trace_cores=None):
    nc = _get_nc()
    return run_bass_kernel_spmd(nc, in_maps, list(range(NCORES)),
                                trace=trace, trace_cores=trace_cores)


def kernel(x, y, w_qkv1, w_qkv2, w_p1, b_p1, w_p2, b_p2):
    x = np.asarray(x, dtype=np.float32)
    y = np.asarray(y, dtype=np.float32)
    in_maps = make_in_maps(x, y, np.asarray(w_qkv1), np.asarray(w_qkv2),
                           np.asarray(w_p1), np.asarray(w_p2))
    res = run_cores(in_maps).results
    out1 = np.stack([res[2 * b]["p1"] + res[2 * b + 1]["p1"] for b in range(4)])
    out2 = np.stack([res[2 * b]["p2"] + res[2 * b + 1]["p2"] for b in range(4)])
    out1 += np.asarray(b_p1, dtype=np.float32)
    out2 += np.asarray(b_p2, dtype=np.float32)
    return out1, out2


# revision 47
# speedup vs baseline: 1.0088x; 1.0088x over previous
"""Trainium2 Bass kernel for the dual-branch cross-attention module.

Computation (see the module's reference):
    q1,k1,v1 = split(x @ w_qkv1); q2,k2,v2 = split(y @ w_qkv2)   (B,H,L,D)
    a1 = softmax(1 - q1 k2^T / sqrt(D));  xo = a1 @ v1
    a2 = softmax(1 - q2 k1^T / sqrt(D));  yo = a2 @ v2
    out = (xo @ w_p1 + b_p1, yo @ w_p2 + b_p2)

Sharding: batch*heads across 8 cores. Core c handles batch b=c//2 and the
8-head slice h0=(c%2)*8. Each core computes its full LxL attention and a
partial output projection over its 512 channels; the host sums the two
partials per batch and adds the bias (softmax(1-z) == softmax(-z), so the
constant shift is dropped).

Self-contained: shapes/sharding hardcoded; imports only the system bass stack.
"""

import os
import sys
from contextlib import ExitStack

import numpy as np

for _p in ("/opt/trn_rl_repo", os.path.expanduser("~/.axon_site/_ro/trn_rl_repo")):
    if os.path.isdir(_p) and _p not in sys.path:
        sys.path.insert(0, _p)

import concourse.tile as tile
from concourse import bacc, mybir
from concourse.bass_utils import run_bass_kernel_spmd
from concourse.masks import make_identity

F32 = mybir.dt.float32
BF16 = mybir.dt.bfloat16
EXP = mybir.ActivationFunctionType.Exp

L = 1024          # sequence length
DIM = 1024        # model dim
D = 64            # head dim
SCALE = D ** -0.5
PROJ = 256        # projection out dim
NCORES = 8
PAIRS = 4         # head pairs per core (8 heads / 2)
KC = 8            # contraction chunks of 128 over DIM
MC = 8            # key-position chunks of 128 over L
LWIN = 512        # query-window (matmul free dim)
NLW = L // LWIN

W_NAMES = ("wq1", "wk1", "wv1", "wq2", "wk2", "wv2")  # x-side first (load order)


def _build_body(nc, tc, ins, outs, ctx):
    stage = ctx.enter_context(tc.tile_pool(name="stage", bufs=3))
    big = ctx.enter_context(tc.tile_pool(name="big", bufs=1))
    qkp = ctx.enter_context(tc.tile_pool(name="qkp", bufs=3))
    vpool = ctx.enter_context(tc.tile_pool(name="vp", bufs=1))
    ep = ctx.enter_context(tc.tile_pool(name="ep", bufs=5))
    onp = ctx.enter_context(tc.tile_pool(name="onp", bufs=1))
    smp = ctx.enter_context(tc.tile_pool(name="smp", bufs=3))
    outp = ctx.enter_context(tc.tile_pool(name="outp", bufs=3))
    mm_ps = ctx.enter_context(tc.tile_pool(name="mm_ps", bufs=2, space="PSUM"))
    st_ps = ctx.enter_context(tc.tile_pool(name="st_ps", bufs=2, space="PSUM"))
    pv_ps = ctx.enter_context(tc.tile_pool(name="pv_ps", bufs=2, space="PSUM"))

    # ---- load x/y, transpose on PE (fp32), cast to bf16 on the psum copy ----
    # xT[p, c, l] = x[l, c*128 + p]
    ident = big.tile([128, 128], BF16, tag="ident")
    make_identity(nc, ident)
    xT = big.tile([128, KC, L], BF16, tag="xT")
    yT = big.tile([128, KC, L], BF16, tag="yT")
    w_bf = {}

    def emit_w_load(nm):
        wt = big.tile([128, KC, 512], BF16, tag=nm)
        w_r = ins[nm].rearrange("(c p) n -> p c n", p=128)
        for c in range(KC):
            wst = stage.tile([128, 512], F32, tag="wst")
            nc.sync.dma_start(out=wst, in_=w_r[:, c, :])
            nc.vector.tensor_copy(out=wt[:, c, :], in_=wst)
        w_bf[nm] = wt

    for src_name, dstT in (("xb", xT), ("yb", yT)):
        src_r = ins[src_name].rearrange("(i p) d -> p i d", p=128)
        for i in range(L // 128):
            xst = stage.tile([128, DIM], F32, tag="xst")
            nc.sync.dma_start(out=xst, in_=src_r[:, i, :])
            xbf = stage.tile([128, DIM], BF16, tag="xbf")
            nc.scalar.copy(out=xbf, in_=xst)
            for j in range(KC):
                tp = mm_ps.tile([128, 512], BF16, tag="mm")
                nc.tensor.transpose(tp[:, 0:128], xbf[:, j * 128:(j + 1) * 128],
                                    ident)
                nc.vector.tensor_copy(out=dstT[:, j, i * 128:(i + 1) * 128],
                                      in_=tp[:, 0:128])
        if src_name == "xb":
            for nm in ("wq1", "wk1", "wv1"):
                emit_w_load(nm)

    # ---- remaining weights ----
    for nm in ("wq2", "wk2", "wv2"):
        emit_w_load(nm)
    wp_bf = {}
    for nm in ("wp1", "wp2"):
        wt = big.tile([128, PAIRS, PROJ], BF16, tag=nm)
        w_r = ins[nm].rearrange("(c p) n -> p c n", p=128)
        for c in range(PAIRS):
            wst = stage.tile([128, 512], F32, tag="wst")
            nc.sync.dma_start(out=wst[:, 0:PROJ], in_=w_r[:, c, :])
            nc.vector.tensor_copy(out=wt[:, c, :], in_=wst[:, 0:PROJ])
        wp_bf[nm] = wt

    onorm = {}  # (pair, branch) -> [128, L] bf16, rows 0:64 head A, 64:128 head B
    vaug = {}   # (pair, branch) -> [128, MC, 130] bf16

    def emit_qk(p):
        cols = slice(p * 128, (p + 1) * 128)
        qk = {}
        for nm, src in (("q1", xT), ("k1", xT), ("q2", yT), ("k2", yT)):
            dstT = qkp.tile([128, L], BF16, tag=nm)
            wt = w_bf["w" + nm]
            for lw in range(NLW):
                mm = mm_ps.tile([128, 512], F32, tag="mm")
                for c in range(KC):
                    nc.tensor.matmul(
                        mm, wt[:, c, cols], src[:, c, lw * LWIN:(lw + 1) * LWIN],
                        start=(c == 0), stop=(c == KC - 1),
                    )
                nc.scalar.copy(out=dstT[:, lw * LWIN:(lw + 1) * LWIN], in_=mm)
            qk[nm] = dstT
        return qk

    def emit_v():
        # v for all pairs: natural layout + ones columns (N=512 matmuls)
        for br, (nm, src) in enumerate((("wv1", xT), ("wv2", yT))):
            wt = w_bf[nm]
            for p in range(PAIRS):
                va = vpool.tile([128, MC, 130], BF16, tag=f"va_{p}_{br}")
                nc.vector.memset(va[:, :, 64:65], 1.0)
                nc.vector.memset(va[:, :, 129:130], 1.0)
                vaug[(p, br)] = va
            for lt in range(MC):
                mm = mm_ps.tile([128, 512], F32, tag="mm")
                for c in range(KC):
                    nc.tensor.matmul(
                        mm, src[:, c, lt * 128:(lt + 1) * 128], wt[:, c, :],
                        start=(c == 0), stop=(c == KC - 1),
                    )
                for p in range(PAIRS):
                    va = vaug[(p, br)]
                    nc.vector.tensor_copy(out=va[:, lt, 0:64],
                                          in_=mm[:, p * 128:p * 128 + 64])
                    nc.vector.tensor_copy(out=va[:, lt, 65:129],
                                          in_=mm[:, p * 128 + 64:(p + 1) * 128])

    def emit_proj(br):
        # partial output projection over this core's 512 channels
        wp_nm, out_nm = (("wp1", "p1"), ("wp2", "p2"))[br]
        wt = wp_bf[wp_nm]
        out_r = outs[out_nm].rearrange("(i p) n -> p i n", p=128)
        for lt in range(L // 128):
            tsl = slice(lt * 128, (lt + 1) * 128)
            mm = mm_ps.tile([128, 512], F32, tag="mm")
            for pp in range(PAIRS):
                nc.tensor.matmul(mm[:, 0:PROJ], onorm[(pp, br)][:, tsl],
                                 wt[:, pp, :], start=(pp == 0), stop=(pp == PAIRS - 1))
            ob = outp.tile([128, PROJ], F32, tag="ob")
            nc.vector.tensor_copy(out=ob, in_=mm[:, 0:PROJ])
            nc.sync.dma_start(out=out_r[:, lt, :], in_=ob)

    for p in range(PAIRS):
        qk = emit_qk(p)
        if p == 0:
            emit_v()

        # ---- attention, both branches ----
        for br in range(2):
            qT = qk["q1"] if br == 0 else qk["q2"]
            kT = qk["k2"] if br == 0 else qk["k1"]
            va = vaug[(p, br)]
            on = onp.tile([128, L], BF16, tag=f"on_{p}_{br}")
            onorm[(p, br)] = on
            for lw in range(NLW):
                lsl = slice(lw * LWIN, (lw + 1) * LWIN)
                pvA = pv_ps.tile([65, 512], F32, tag="pv")
                pvB = pv_ps.tile([65, 512], F32, tag="pv")
                for mc in range(MC):
                    msl = slice(mc * 128, (mc + 1) * 128)
                    st = st_ps.tile([128, 1024], F32, tag="st")
                    # S^T[m, l] for both heads (K=64, row-packed 0-63 / 64-127)
                    nc.tensor.matmul(st[:, 0:512], kT[0:64, msl], qT[0:64, lsl],
                                     start=True, stop=True)
                    nc.tensor.matmul(st[:, 512:1024], kT[64:128, msl],
                                     qT[64:128, lsl], start=True, stop=True)
                    # E = exp(-S/sqrt(D)); softmax(1-z) == softmax(-z)
                    e_t = ep.tile([128, 1024], BF16, tag="E")
                    nc.scalar.activation(out=e_t, in_=st, func=EXP, scale=-SCALE)
                    # O^T accum + row-sums via the ones column
                    nc.tensor.matmul(pvA, va[:, mc, 0:65], e_t[:, 0:512],
                                     start=(mc == 0), stop=(mc == MC - 1))
                    nc.tensor.matmul(pvB, va[:, mc, 65:130], e_t[:, 512:1024],
                                     start=(mc == 0), stop=(mc == MC - 1))
                # normalize: O^T * (1/rowsum) broadcast along partitions
                for head, pv in ((0, pvA), (1, pvB)):
                    ssum = smp.tile([1, 512], F32, tag="ssum")
                    nc.vector.tensor_copy(out=ssum, in_=pv[64:65, :])
                    pvo = smp.tile([64, 512], F32, tag="pvo")
                    nc.vector.tensor_copy(out=pvo, in_=pv[0:64, :])
                    sb = smp.tile([64, 512], F32, tag="sb")
                    nc.gpsimd.partition_broadcast(sb, ssum)
                    rb = smp.tile([64, 512], F32, tag="rb")
                    nc.vector.reciprocal_approx_fast(out=rb, in_=sb)
                    nc.vector.tensor_mul(
                        out=on[head * 64:(head + 1) * 64, lsl],
                        in0=pvo, in1=rb,
                    )



    # ---- output projection (partial over this core's 512 channels) ----
    for br in range(2):
        emit_proj(br)

def build():
    nc = bacc.Bacc("TRN2", target_bir_lowering=False, debug=False,
                   num_devices=NCORES)
    ins = {}
    for nm in ("xb", "yb"):
        ins[nm] = nc.dram_tensor(nm, [L, DIM], F32, kind="ExternalInput").ap()
    for nm in W_NAMES:
        ins[nm] = nc.dram_tensor(nm, [DIM, 512], F32, kind="ExternalInput").ap()
    for nm in ("wp1", "wp2"):
        ins[nm] = nc.dram_tensor(nm, [512, PROJ], F32, kind="ExternalInput").ap()
    outs = {}
    for nm in ("p1", "p2"):
        outs[nm] = nc.dram_tensor(nm, [L, PROJ], F32, kind="ExternalOutput").ap()
    with tile.TileContext(nc) as tc:
        with ExitStack() as ctx:
            _build_body(nc, tc, ins, outs, ctx)
    nc.compile()
    return nc


_NC_CACHE = None


def _get_nc():
    global _NC_CACHE
    if _NC_CACHE is None:
        _NC_CACHE = build()
    return _NC_CACHE


def make_in_maps(x, y, w_qkv1, w_qkv2, w_p1, w_p2):
    """Shard the full inputs: core c -> batch c//2, head-slice (c%2)*8."""
    in_maps = []
    for c in range(NCORES):
        b, half = divmod(c, 2)
        c0 = half * 512  # channel offset of this core's 8 heads
        m = {
            "xb": np.ascontiguousarray(x[b]),
            "yb": np.ascontiguousarray(y[b]),
            "wp1": np.ascontiguousarray(w_p1[c0:c0 + 512, :]),
            "wp2": np.ascontiguousarray(w_p2[c0:c0 + 512, :]),
        }
        for wsrc, names in ((w_qkv1, ("wq1", "wk1", "wv1")),
                            (w_qkv2, ("wq2", "wk2", "wv2"))):
            for j, nm in enumerate(names):
                base = j * DIM + c0
                m[nm] = np.ascontiguousarray(wsrc[:, base:base + 512])
        in_maps.append(m)
    return in_maps


def run_cores(in_maps, trace=False, trace_cores=None):
    nc = _get_nc()
    return run_bass_kernel_spmd(nc, in_maps, list(range(NCORES)),
                                trace=trace, trace_cores=trace_cores)


def kernel(x, y, w_qkv1, w_qkv2, w_p1, b_p1, w_p2, b_p2):
    x = np.asarray(x, dtype=np.float32)
    y = np.asarray(y, dtype=np.float32)
    in_maps = make_in_maps(x, y, np.asarray(w_qkv1), np.asarray(w_qkv2),
                           np.asarray(w_p1), np.asarray(w_p2))
    res = run_cores(in_maps).results
    out1 = np.stack([res[2 * b]["p1"] + res[2 * b + 1]["p1"] for b in range(4)])
    out2 = np.stack([res[2 * b]["p2"] + res[2 * b + 1]["p2"] for b in range(4)])
    out1 += np.asarray(b_p1, dtype=np.float32)
    out2 += np.asarray(b_p2, dtype=np.float32)
    return out1, out2
